# revision 62
# baseline (speedup 1.0000x reference)
"""Trainium2 Bass kernel for multiplicative (Bahdanau-style) attention.

Computes, for inputs encoder_hidden [B,T,H], decoder_hidden [B,H] and small
params W1,b1,W2,b2,V,bV:
    dec_w   = decoder_hidden @ W1 + b1                  # [B, LD]
    enc_w   = encoder_hidden @ W2 + b2                  # [B, T, LD]
    score   = tanh(dec_w[:,None,:] * enc_w) @ V + bV    # [B, T, 1]
    attn    = softmax(score, axis=1)
    context = sum(attn * encoder_hidden, axis=1)        # [B, H]
    returns (context, attn)

Sharding: data-parallel over batch B=32 across 8 NeuronCores (4 per core).
Params replicated.  Per core the dominant work is the [T,H]@[H,LD] matmul
per batch, done on the TensorEngine in bf16 with the encoder tensor fed in
[h, t] layout (host-pretransposed, bf16 wire format) so the contraction
dim lands on SBUF partitions.  enc_w lives on chip as [l, t], which makes
the dec_w multiply a per-partition scale fused into the ScalarEngine tanh
(b2 folded into the bias).  The @V score reduction packs the four t-quarter
M=1 matvecs into four 32-column groups of the PE array (tile_position) so
they run concurrently, accumulating across l-chunks on partitions
0/32/64/96 of one PSUM bank, interleaved two l-chunks behind the mains;
mains use one PSUM bank per t-quarter (6-deep pool) so ScalarE evacuation
releases slots incrementally.  Softmax runs on partition 0 with no
max-subtraction (scores are O(1)); exp runs on ScalarE straight from PSUM
with fused sum accumulation.  The unnormalized attn row is broadcast to
128 partitions by a K=1 PE ones-matmul, the context reduction is a
single-pass DVE multiply+accumulate per h-chunk in bf16, and the 1/sum
scale lands on the tiny context result via an exact f32 DRAM-round-trip
broadcast hidden under the reduction.  Each batch's softmax/context
epilogue is deferred into the next batch's PE stream so only the last one
is exposed.  b1 enters dec_w via an extra K=1 matmul row; bV cancels in
the softmax and is ignored.  Small tensors are pre-arranged on the host
into their on-chip [partition, free] layouts to avoid scattered DMA
descriptors at kernel start.
"""

import sys

for _p in ("/opt/trn_rl_repo",):
    if _p not in sys.path:
        sys.path.insert(0, _p)

from contextlib import ExitStack

import numpy as np


def _install_ntff_hook_shim():
    """The image's ``antenv`` lacks ``axon_hooks``; provide it so
    ``run_bass_kernel_spmd(trace=True)`` can profile via the axon .so."""
    import types

    try:
        from antenv.axon_hooks import get_axon_ntff_profile_hook  # noqa: F401
        return
    except ImportError:
        pass
    try:
        import antenv

        mod = types.ModuleType("antenv.axon_hooks")
        holder = {"hook": None}
        mod.set_axon_ntff_profile_hook = lambda h: holder.__setitem__("hook", h)
        mod.get_axon_ntff_profile_hook = lambda: holder["hook"]
        sys.modules["antenv.axon_hooks"] = mod
        antenv.axon_hooks = mod
        from trn_agent_boot.trn_boot import _ntff_profile_via_ctypes

        hook = _ntff_profile_via_ctypes("/opt/axon/libaxon_pjrt.so")
        if hook is not None:
            mod.set_axon_ntff_profile_hook(hook)
    except Exception:
        pass


_install_ntff_hook_shim()

import concourse.bass as bass
import concourse.mybir as mybir
from concourse import bass2jax as _bass2jax
from concourse import bass_utils as _bass_utils
from concourse.bass_utils import run_bass_kernel_spmd
from concourse.tile import TileContext


def _split_excess_waits(bir_json: bytes, max_waits: int = 1) -> bytes:
    """This image's walrus rejects instructions carrying more than one sem
    wait ("Too many sync wait commands"), which Tile-generated kernels do.
    Move excess waits onto same-engine NoOp carriers just before the
    instruction — semantically identical, walrus-acceptable."""
    import json as _json

    j = _json.loads(bir_json)
    for fn in j["functions"]:
        for bb in fn["blocks"]:
            out = []
            for inst in bb["instructions"]:
                si = inst.get("sync_info")
                ow = (si or {}).get("on_wait") or []
                if len(ow) > max_waits:
                    extra, keep = ow[:-max_waits], ow[-max_waits:]
                    inst["sync_info"]["on_wait"] = keep
                    for gi in range(0, len(extra), max_waits):
                        out.append(
                            {
                                "debug": inst.get("debug", 0),
                                "engine": inst["engine"],
                                "ins": [],
                                "outs": [],
                                "name": f"{inst['name']}_xw{gi}",
                                "opcode": "NoOp",
                                "sync_info": {
                                    "on_update": [],
                                    "on_wait": extra[gi : gi + max_waits],
                                },
                            }
                        )
                out.append(inst)
            bb["instructions"] = out
    return _json.dumps(j).encode()


def _patch_compiler_for_wait_limit():
    if getattr(_bass2jax, "_wait_split_patched", False):
        return
    orig = _bass_utils.compile_bir_kernel

    def patched(bir_json, tmpdir, neff_name="file.neff"):
        return orig(_split_excess_waits(bir_json), tmpdir, neff_name)

    _bass2jax.compile_bir_kernel = patched
    _bass2jax._wait_split_patched = True


_patch_compiler_for_wait_limit()

B, T, H, LD = 32, 2048, 1024, 1024
NCORES = 8
BL = B // NCORES          # batches per core
HC = H // 128             # h chunks
LC = LD // 128            # l chunks
NT = T // 512             # 512-wide t quarters

F32 = mybir.dt.float32
BF16 = mybir.dt.bfloat16
AF = mybir.ActivationFunctionType
ALU = mybir.AluOpType
AX = mybir.AxisListType

_CACHE = {}


def _build():
    nc = bass.Bass()
    enc_t = nc.dram_tensor("enc_t", [BL, H, T], BF16, kind="ExternalInput")
    dec_p = nc.dram_tensor("dec_p", [128, HC * BL], BF16, kind="ExternalInput")
    w1 = nc.dram_tensor("w1", [H, LD], BF16, kind="ExternalInput")
    w2 = nc.dram_tensor("w2", [H, LD], BF16, kind="ExternalInput")
    b1 = nc.dram_tensor("b1", [LD], BF16, kind="ExternalInput")
    b2t = nc.dram_tensor("b2t", [128, LC], F32, kind="ExternalInput")
    v = nc.dram_tensor("v", [128, LC], BF16, kind="ExternalInput")
    out_ctx = nc.dram_tensor("out_ctx", [BL, H], F32, kind="ExternalOutput")
    out_attn = nc.dram_tensor("out_attn", [BL, T], F32, kind="ExternalOutput")

    with ExitStack() as ctx:
        tc = ctx.enter_context(TileContext(nc))
        singles = ctx.enter_context(tc.tile_pool(name="singles", bufs=1))
        encp = ctx.enter_context(tc.tile_pool(name="encp", bufs=2 * HC))
        tanhp = ctx.enter_context(tc.tile_pool(name="tanhp", bufs=HC + 2))
        bigp = ctx.enter_context(tc.tile_pool(name="bigp", bufs=3))
        rowp = ctx.enter_context(tc.tile_pool(name="rowp", bufs=2))
        bcp = ctx.enter_context(tc.tile_pool(name="bcp", bufs=2))
        smallp = ctx.enter_context(tc.tile_pool(name="smallp", bufs=8))
        dramp = ctx.enter_context(tc.tile_pool(name="dramp", bufs=2, space="DRAM"))
        psmm = ctx.enter_context(tc.tile_pool(name="psmm", bufs=6, space="PSUM"))
        pssc = ctx.enter_context(tc.tile_pool(name="pssc", bufs=2, space="PSUM"))

        # --- persistent params (bf16 wire format, no casts, HWDGE) ---
        # Load order: W1 + decoder first (dec_w prologue is first in the PE
        # queue), then W2, then batch-0 encoder chunks (per-chunk DMAs so
        # batch-0 mains start progressively as chunks land).
        w1_sb = singles.tile([128, HC, LD], BF16, tag="w1")
        nc.sync.dma_start(out=w1_sb, in_=w1.rearrange("(hc p) l -> p hc l", p=128))
        dec_sb = singles.tile([128, HC, BL], BF16, tag="dec")
        nc.sync.dma_start(out=dec_sb, in_=dec_p.rearrange("p (hc b) -> p hc b", hc=HC))
        b1_sb = singles.tile([1, LC, 128], BF16, tag="b1")
        nc.sync.dma_start(
            out=b1_sb, in_=b1.rearrange("(one lc m) -> one lc m", one=1, m=128)
        )
        v_sb = singles.tile([128, LC], BF16, tag="v")
        nc.sync.dma_start(out=v_sb, in_=v[:, :])
        b2t_sb = singles.tile([128, LC], F32, tag="b2t")
        nc.sync.dma_start(out=b2t_sb, in_=b2t[:, :])
        ones_sb = singles.tile([1, BL], BF16, tag="ones")
        nc.vector.memset(ones_sb, 1.0)
        dec_w = singles.tile([128, LC, BL], F32, tag="dec_w")
        db2 = singles.tile([128, LC, BL], F32, tag="db2")

        w2_sb = singles.tile([128, HC, LD], BF16, tag="w2")
        nc.sync.dma_start(out=w2_sb, in_=w2.rearrange("(hc p) l -> p hc l", p=128))
        enc0_tiles = []
        for hc in range(HC):
            et = encp.tile([128, T], BF16, tag="enc", name=f"enc_b0_h{hc}")
            nc.sync.dma_start(out=et, in_=enc_t[0, hc * 128 : (hc + 1) * 128, :])
            enc0_tiles.append(et)

        def emit_dec_w():
            # dec_w[l, b] = sum_h W1[h,l] dec[h,b] + b1[l]
            for lc in range(LC):
                ps = pssc.tile([128, BL], F32, tag="score", name=f"dw_ps_{lc}")
                for hc in range(HC):
                    nc.tensor.matmul(
                        ps,
                        lhsT=w1_sb[:, hc, lc * 128 : (lc + 1) * 128],
                        rhs=dec_sb[:, hc, :],
                        start=(hc == 0),
                        stop=False,
                    )
                nc.tensor.matmul(
                    ps, lhsT=b1_sb[:, lc, :], rhs=ones_sb, start=False, stop=True
                )
                nc.vector.tensor_copy(dec_w[:, lc, :], ps)
            # db2[l, b] = dec_w[l, b] * b2[l]  (tanh bias after distributing dec_w)
            for b in range(BL):
                nc.vector.tensor_mul(db2[:, :, b], dec_w[:, :, b], b2t_sb)

        emit_dec_w()

        ones128 = singles.tile([1, 128], BF16, tag="ones128")
        nc.vector.memset(ones128, 1.0)

        def emit_score(b, lc, score128, tanh_tiles):
            # score[t] += V[lc-chunk] . tanh[lc-chunk, t] — the four t-quarter
            # matvecs are packed into four 32-column groups of the PE array
            # via tile_position, so they run concurrently (~4x faster than
            # serial M=1 matmuls); results land on partitions 0/32/64/96 of
            # one PSUM bank
            for tq in range(NT):
                nc.tensor.matmul(
                    score128[32 * tq : 32 * tq + 1, :],
                    lhsT=v_sb[:, lc : lc + 1],
                    rhs=tanh_tiles[lc][:, tq * 512 : (tq + 1) * 512],
                    start=(lc == 0),
                    stop=(lc == LC - 1),
                    tile_position=(0, 32 * tq),
                    skip_group_check=True,
                )

        def emit_softmax(b, score128, tanh_tiles):
            # remaining score chunks
            emit_score(b, LC - 2, score128, tanh_tiles)
            emit_score(b, LC - 1, score128, tanh_tiles)

            # --- softmax over T (partition 0; exp on ScalarE) ---
            # No max subtraction: scores are O(1) sums against V ~ N(0,1/LD),
            # so exp cannot overflow and softmax(s) == softmax(s - max)
            # exactly. Each exp starts as soon as its score quarter stops.
            arow = rowp.tile([1, T], BF16, tag="arow", name=f"arow_{b}")
            sm4 = smallp.tile([1, NT], F32, tag="sm4", name=f"sm4_{b}")
            for tq in range(NT):
                sl = slice(tq * 512, (tq + 1) * 512)
                nc.scalar.activation(
                    out=arow[0:1, sl],
                    in_=score128[32 * tq : 32 * tq + 1, :],
                    func=AF.Exp,
                    bias=0.0,
                    scale=1.0,
                    accum_out=sm4[0:1, tq : tq + 1],
                )
            ssum = smallp.tile([1, 1], F32, tag="ssum", name=f"ssum_{b}")
            nc.vector.reduce_sum(out=ssum, in_=sm4, axis=AX.X)
            rsum = smallp.tile([1, 1], F32, tag="rsum", name=f"rsum_{b}")
            nc.vector.reciprocal(rsum, ssum)
            return arow, rsum

        def emit_context(b, arow, rsum, enc_tiles):
            # normalized f32 attn row for the output (ScalarE, off the
            # critical path — runs while DVE does the context reduction)
            anorm = rowp.tile([1, T], F32, tag="anorm", name=f"anorm_{b}")
            nc.scalar.activation(
                out=anorm[0:1, :], in_=arow[0:1, :], func=AF.Copy,
                scale=rsum[0:1, :],
            )
            nc.sync.dma_start(out=out_attn[b : b + 1, :], in_=anorm[0:1, :])

            # --- context[h] = sum_t attn[t] * enc[h, t] ---
            # broadcast the UNnormalized attn row to 128 partitions on the
            # PE (ones.T @ arow — starts as soon as each exp quarter lands);
            # the 1/sum scale is applied to the tiny context result at the
            # end, using an exact f32 broadcast of rsum via a DRAM
            # round-trip that hides under the context reduction
            rs_dram = dramp.tile([1, 1], F32, tag="rs_dram", name=f"rs_dram_{b}")
            nc.sync.dma_start(out=rs_dram, in_=rsum)
            rs_bc = smallp.tile([128, 1], F32, tag="rs_bc", name=f"rs_bc_{b}")
            nc.sync.dma_start(out=rs_bc, in_=rs_dram[0:1, :].partition_broadcast(128))
            abc_ps = []
            for tq in range(NT):
                aps = pssc.tile(
                    [128, 512], F32, tag="score", name=f"abc_ps_{b}_{tq}"
                )
                nc.tensor.matmul(
                    aps,
                    lhsT=ones128,
                    rhs=arow[0:1, tq * 512 : (tq + 1) * 512],
                    start=True,
                    stop=True,
                )
                abc_ps.append(aps)
            abc = bcp.tile([128, T], BF16, tag="abc", name=f"abc_{b}")
            for tq in range(NT):
                nc.vector.tensor_copy(
                    abc[:, tq * 512 : (tq + 1) * 512], abc_ps[tq]
                )
            ctxs = smallp.tile([128, HC], F32, tag="ctx", name=f"ctxs_{b}")
            for hc in range(HC):
                # scr = (enc bypass) * attn_u ; ctxs[:,hc] = sum_t scr
                scr = bigp.tile([128, T], BF16, tag="big", name=f"scr_{b}_{hc}")
                nc.vector.scalar_tensor_tensor(
                    out=scr,
                    in0=enc_tiles[hc],
                    scalar=1.0,
                    in1=abc,
                    op0=ALU.bypass,
                    op1=ALU.mult,
                    accum_out=ctxs[:, hc : hc + 1],
                )
            nc.vector.tensor_scalar_mul(ctxs, ctxs, rs_bc[:, 0:1])
            nc.sync.dma_start(
                out=out_ctx[b : b + 1, :].rearrange("one (hc p) -> p (one hc)", p=128),
                in_=ctxs,
            )

        # --- main per-batch pipeline ---
        pending_softmax = None
        pending_context = None
        pending_context_args = None
        for b in range(BL):
            if b == 0:
                enc_tiles = enc0_tiles
            else:
                enc_tiles = []
                for hc in range(HC):
                    et = encp.tile([128, T], BF16, tag="enc", name=f"enc_b{b}_h{hc}")
                    nc.sync.dma_start(
                        out=et, in_=enc_t[b, hc * 128 : (hc + 1) * 128, :]
                    )
                    enc_tiles.append(et)

            score128 = None
            # main matmuls: enc_w[l, t] accumulated over h; one weight load
            # (W2 h,l block) covers 4 moving matmuls.  One PSUM bank per
            # t-quarter so ScalarE evacuation releases slots incrementally.
            tanh_tiles = []
            for lc in range(LC):
                psq = [
                    psmm.tile([128, 512], F32, tag="encw", name=f"encw_b{b}_l{lc}q{q}")
                    for q in range(4)
                ]
                for hc in range(HC):
                    for q in range(4):
                        nc.tensor.matmul(
                            psq[q],
                            lhsT=w2_sb[:, hc, lc * 128 : (lc + 1) * 128],
                            rhs=enc_tiles[hc][:, q * 512 : (q + 1) * 512],
                            start=(hc == 0),
                            stop=(hc == HC - 1),
                        )
                if lc == 0 and pending_softmax is not None:
                    # previous batch's softmax: emitted here so its exp runs
                    # on ScalarE right after that batch's own tanh quarters,
                    # before this batch's tanh
                    pending_context_args = pending_softmax()
                    pending_softmax = None
                if lc == 1 and pending_context is not None:
                    # previous batch's broadcast + context: by now its exp
                    # results are ready, so the PE broadcast doesn't stall
                    pending_context(*pending_context_args)
                    pending_context = None
                th = tanhp.tile([128, T], BF16, tag="tanh", name=f"tanh_b{b}_l{lc}")
                tanh_tiles.append(th)
                for q in range(4):
                    # tanh(dec_w * enc_w + dec_w*b2), per-partition scale/bias
                    nc.scalar.activation(
                        out=th[:, q * 512 : (q + 1) * 512],
                        in_=psq[q],
                        func=AF.Tanh,
                        scale=dec_w[:, lc, b : b + 1],
                        bias=db2[:, lc, b : b + 1],
                    )
                if lc >= 2:
                    # scores for lc-2 (its tanh is ready by now); the score
                    # PSUM tile is allocated lazily here so its slot isn't
                    # held across the previous batch's deferred epilogue
                    if score128 is None:
                        score128 = pssc.tile(
                            [128, 512], F32, tag="score", name=f"score_b{b}"
                        )
                    emit_score(b, lc - 2, score128, tanh_tiles)

            pending_softmax = (
                lambda b=b, sp=score128, tt=tanh_tiles:
                emit_softmax(b, sp, tt)
            )
            def _mk_ctx(b=b, et=enc_tiles):
                return lambda arow, rsum: emit_context(b, arow, rsum, et)
            pending_context = _mk_ctx()
        pending_context_args = pending_softmax()
        pending_context(*pending_context_args)
    return nc


def _get_nc():
    if "nc" not in _CACHE:
        _CACHE["nc"] = _build()
    return _CACHE["nc"]


def kernel(encoder_hidden, decoder_hidden, W1, b1, W2, b2, V, bV, _trace=False):
    encoder_hidden = np.asarray(encoder_hidden, dtype=np.float32)
    decoder_hidden = np.asarray(decoder_hidden, dtype=np.float32)
    W1 = np.asarray(W1, dtype=np.float32)
    W2 = np.asarray(W2, dtype=np.float32)
    b1 = np.asarray(b1, dtype=np.float32)
    b2 = np.asarray(b2, dtype=np.float32)
    V = np.asarray(V, dtype=np.float32)

    # host-side prep: relayout + bf16 wire format (the device computes in
    # bf16 either way — this is the same rounding the on-device DMA cast
    # performed, just done host-side so the wire bytes halve)
    import ml_dtypes

    bf16 = ml_dtypes.bfloat16
    enc_t_full = np.ascontiguousarray(
        encoder_hidden.transpose(0, 2, 1).astype(bf16)
    )  # [B, H, T] bf16
    b2t = np.ascontiguousarray(b2.reshape(LC, 128).T)                     # [128, LC]
    W1b = W1.astype(bf16)
    W2b = W2.astype(bf16)
    b1b = b1.astype(bf16)
    # v and decoder pre-arranged into their on-chip [partition, free]
    # layouts so the loads are clean contiguous descriptors
    Vp = np.ascontiguousarray(V.reshape(LC, 128).T.astype(bf16))          # [128, LC]

    in_maps = []
    for c in range(NCORES):
        sl = slice(c * BL, (c + 1) * BL)
        in_maps.append(
            {
                "enc_t": np.ascontiguousarray(enc_t_full[sl]),
                "dec_p": np.ascontiguousarray(
                    decoder_hidden[sl]                       # [BL, H]
                    .T.reshape(HC, 128, BL)                  # [hc, p, b]
                    .transpose(1, 0, 2)                      # [p, hc, b]
                    .reshape(128, HC * BL)
                    .astype(bf16)
                ),
                "w1": W1b,
                "w2": W2b,
                "b1": b1b,
                "b2t": b2t,
                "v": Vp,
            }
        )

    nc = _get_nc()
    res = run_bass_kernel_spmd(nc, in_maps, core_ids=list(range(NCORES)), trace=_trace)
    if _trace:
        _CACHE["last_result"] = res

    context = np.concatenate([r["out_ctx"] for r in res.results], axis=0)  # [B, H]
    attn = np.concatenate([r["out_attn"] for r in res.results], axis=0)    # [B, T]
    return context.astype(np.float32), attn.reshape(B, T, 1).astype(np.float32)


# revision 63
# speedup vs baseline: 1.0275x; 1.0275x over previous
"""Trainium2 Bass kernel for multiplicative (Bahdanau-style) attention.

Computes, for inputs encoder_hidden [B,T,H], decoder_hidden [B,H] and small
params W1,b1,W2,b2,V,bV:
    dec_w   = decoder_hidden @ W1 + b1                  # [B, LD]
    enc_w   = encoder_hidden @ W2 + b2                  # [B, T, LD]
    score   = tanh(dec_w[:,None,:] * enc_w) @ V + bV    # [B, T, 1]
    attn    = softmax(score, axis=1)
    context = sum(attn * encoder_hidden, axis=1)        # [B, H]
    returns (context, attn)

Sharding: data-parallel over batch B=32 across 8 NeuronCores (4 per core).
Params replicated.  Per core the dominant work is the [T,H]@[H,LD] matmul
per batch, done on the TensorEngine in bf16 with the encoder tensor fed in
[h, t] layout (host-pretransposed, bf16 wire format) so the contraction
dim lands on SBUF partitions.  enc_w lives on chip as [l, t], which makes
the dec_w multiply a per-partition scale fused into the ScalarEngine tanh
(b2 folded into the bias).  The @V score reduction packs the four t-quarter
M=1 matvecs into four 32-column groups of the PE array (tile_position) so
they run concurrently, accumulating across l-chunks on partitions
0/32/64/96 of one PSUM bank, interleaved two l-chunks behind the mains;
mains use one PSUM bank per t-quarter (6-deep pool) so ScalarE evacuation
releases slots incrementally.  Softmax runs on partition 0 with no
max-subtraction (scores are O(1)); exp runs on ScalarE straight from PSUM
with fused sum accumulation.  The unnormalized attn row is broadcast to
128 partitions by a K=1 PE ones-matmul, the context reduction is a
single-pass DVE multiply+accumulate per h-chunk in bf16, and the 1/sum
scale lands on the tiny context result via an exact f32 DRAM-round-trip
broadcast hidden under the reduction.  Each batch's softmax/context
epilogue is deferred into the next batch's PE stream so only the last one
is exposed.  b1 enters dec_w via an extra K=1 matmul row; bV cancels in
the softmax and is ignored.  Small tensors are pre-arranged on the host
into their on-chip [partition, free] layouts to avoid scattered DMA
descriptors at kernel start.
"""

import sys

for _p in ("/opt/trn_rl_repo",):
    if _p not in sys.path:
        sys.path.insert(0, _p)

from contextlib import ExitStack

import numpy as np


def _install_ntff_hook_shim():
    """The image's ``antenv`` lacks ``axon_hooks``; provide it so
    ``run_bass_kernel_spmd(trace=True)`` can profile via the axon .so."""
    import types

    try:
        from antenv.axon_hooks import get_axon_ntff_profile_hook  # noqa: F401
        return
    except ImportError:
        pass
    try:
        import antenv

        mod = types.ModuleType("antenv.axon_hooks")
        holder = {"hook": None}
        mod.set_axon_ntff_profile_hook = lambda h: holder.__setitem__("hook", h)
        mod.get_axon_ntff_profile_hook = lambda: holder["hook"]
        sys.modules["antenv.axon_hooks"] = mod
        antenv.axon_hooks = mod
        from trn_agent_boot.trn_boot import _ntff_profile_via_ctypes

        hook = _ntff_profile_via_ctypes("/opt/axon/libaxon_pjrt.so")
        if hook is not None:
            mod.set_axon_ntff_profile_hook(hook)
    except Exception:
        pass


_install_ntff_hook_shim()

import concourse.bass as bass
import concourse.mybir as mybir
from concourse import bass2jax as _bass2jax
from concourse import bass_utils as _bass_utils
from concourse.bass_utils import run_bass_kernel_spmd
from concourse.tile import TileContext


def _split_excess_waits(bir_json: bytes, max_waits: int = 1) -> bytes:
    """This image's walrus rejects instructions carrying more than one sem
    wait ("Too many sync wait commands"), which Tile-generated kernels do.
    Move excess waits onto same-engine NoOp carriers just before the
    instruction — semantically identical, walrus-acceptable."""
    import json as _json

    j = _json.loads(bir_json)
    for fn in j["functions"]:
        for bb in fn["blocks"]:
            out = []
            for inst in bb["instructions"]:
                si = inst.get("sync_info")
                ow = (si or {}).get("on_wait") or []
                if len(ow) > max_waits:
                    extra, keep = ow[:-max_waits], ow[-max_waits:]
                    inst["sync_info"]["on_wait"] = keep
                    for gi in range(0, len(extra), max_waits):
                        out.append(
                            {
                                "debug": inst.get("debug", 0),
                                "engine": inst["engine"],
                                "ins": [],
                                "outs": [],
                                "name": f"{inst['name']}_xw{gi}",
                                "opcode": "NoOp",
                                "sync_info": {
                                    "on_update": [],
                                    "on_wait": extra[gi : gi + max_waits],
                                },
                            }
                        )
                out.append(inst)
            bb["instructions"] = out
    return _json.dumps(j).encode()


def _patch_compiler_for_wait_limit():
    if getattr(_bass2jax, "_wait_split_patched", False):
        return
    orig = _bass_utils.compile_bir_kernel

    def patched(bir_json, tmpdir, neff_name="file.neff"):
        return orig(_split_excess_waits(bir_json), tmpdir, neff_name)

    _bass2jax.compile_bir_kernel = patched
    _bass2jax._wait_split_patched = True


_patch_compiler_for_wait_limit()

B, T, H, LD = 32, 2048, 1024, 1024
NCORES = 8
BL = B // NCORES          # batches per core
HC = H // 128             # h chunks
LC = LD // 128            # l chunks
NT = T // 512             # 512-wide t quarters

F32 = mybir.dt.float32
BF16 = mybir.dt.bfloat16
AF = mybir.ActivationFunctionType
ALU = mybir.AluOpType
AX = mybir.AxisListType

_CACHE = {}


def _build():
    nc = bass.Bass()
    enc_t = nc.dram_tensor("enc_t", [BL, H, T], BF16, kind="ExternalInput")
    dec_p = nc.dram_tensor("dec_p", [128, HC * BL], BF16, kind="ExternalInput")
    w1 = nc.dram_tensor("w1", [H, LD], BF16, kind="ExternalInput")
    w2 = nc.dram_tensor("w2", [H, LD], BF16, kind="ExternalInput")
    b1 = nc.dram_tensor("b1", [LD], BF16, kind="ExternalInput")
    b2t = nc.dram_tensor("b2t", [128, LC], F32, kind="ExternalInput")
    v = nc.dram_tensor("v", [128, LC], BF16, kind="ExternalInput")
    out_ctx = nc.dram_tensor("out_ctx", [BL, 128, HC], F32, kind="ExternalOutput")
    out_attn = nc.dram_tensor("out_attn", [BL, T], F32, kind="ExternalOutput")

    with ExitStack() as ctx:
        tc = ctx.enter_context(TileContext(nc))
        singles = ctx.enter_context(tc.tile_pool(name="singles", bufs=1))
        encp = ctx.enter_context(tc.tile_pool(name="encp", bufs=2 * HC))
        tanhp = ctx.enter_context(tc.tile_pool(name="tanhp", bufs=HC + 2))
        bigp = ctx.enter_context(tc.tile_pool(name="bigp", bufs=3))
        rowp = ctx.enter_context(tc.tile_pool(name="rowp", bufs=2))
        bcp = ctx.enter_context(tc.tile_pool(name="bcp", bufs=2))
        smallp = ctx.enter_context(tc.tile_pool(name="smallp", bufs=8))
        dramp = ctx.enter_context(tc.tile_pool(name="dramp", bufs=2, space="DRAM"))
        psmm = ctx.enter_context(tc.tile_pool(name="psmm", bufs=6, space="PSUM"))
        pssc = ctx.enter_context(tc.tile_pool(name="pssc", bufs=2, space="PSUM"))

        # --- persistent params (bf16 wire format, no casts, HWDGE) ---
        # Load order: W1 + decoder first (dec_w prologue is first in the PE
        # queue), then W2, then batch-0 encoder chunks (per-chunk DMAs so
        # batch-0 mains start progressively as chunks land).
        w1_sb = singles.tile([128, HC, LD], BF16, tag="w1")
        nc.sync.dma_start(out=w1_sb, in_=w1.rearrange("(hc p) l -> p hc l", p=128))
        dec_sb = singles.tile([128, HC, BL], BF16, tag="dec")
        nc.sync.dma_start(out=dec_sb, in_=dec_p.rearrange("p (hc b) -> p hc b", hc=HC))
        b1_sb = singles.tile([1, LC, 128], BF16, tag="b1")
        nc.sync.dma_start(
            out=b1_sb, in_=b1.rearrange("(one lc m) -> one lc m", one=1, m=128)
        )
        v_sb = singles.tile([128, LC], BF16, tag="v")
        nc.sync.dma_start(out=v_sb, in_=v[:, :])
        b2t_sb = singles.tile([128, LC], F32, tag="b2t")
        nc.sync.dma_start(out=b2t_sb, in_=b2t[:, :])
        ones_sb = singles.tile([1, BL], BF16, tag="ones")
        nc.vector.memset(ones_sb, 1.0)
        dec_w = singles.tile([128, LC, BL], F32, tag="dec_w")
        db2 = singles.tile([128, LC, BL], F32, tag="db2")

        w2_sb = singles.tile([128, HC, LD], BF16, tag="w2")
        nc.sync.dma_start(out=w2_sb, in_=w2.rearrange("(hc p) l -> p hc l", p=128))
        enc0_tiles = []
        for hc in range(HC):
            et = encp.tile([128, T], BF16, tag="enc", name=f"enc_b0_h{hc}")
            nc.sync.dma_start(out=et, in_=enc_t[0, hc * 128 : (hc + 1) * 128, :])
            enc0_tiles.append(et)

        def emit_dec_w():
            # dec_w[l, b] = sum_h W1[h,l] dec[h,b] + b1[l]
            for lc in range(LC):
                ps = pssc.tile([128, BL], F32, tag="score", name=f"dw_ps_{lc}")
                for hc in range(HC):
                    nc.tensor.matmul(
                        ps,
                        lhsT=w1_sb[:, hc, lc * 128 : (lc + 1) * 128],
                        rhs=dec_sb[:, hc, :],
                        start=(hc == 0),
                        stop=False,
                    )
                nc.tensor.matmul(
                    ps, lhsT=b1_sb[:, lc, :], rhs=ones_sb, start=False, stop=True
                )
                nc.vector.tensor_copy(dec_w[:, lc, :], ps)
            # db2[l, b] = dec_w[l, b] * b2[l]  (tanh bias after distributing dec_w)
            for b in range(BL):
                nc.vector.tensor_mul(db2[:, :, b], dec_w[:, :, b], b2t_sb)

        emit_dec_w()

        ones128 = singles.tile([1, 128], BF16, tag="ones128")
        nc.vector.memset(ones128, 1.0)

        def emit_score(b, lc, score128, tanh_tiles):
            # score[t] += V[lc-chunk] . tanh[lc-chunk, t] — the four t-quarter
            # matvecs are packed into four 32-column groups of the PE array
            # via tile_position, so they run concurrently (~4x faster than
            # serial M=1 matmuls); results land on partitions 0/32/64/96 of
            # one PSUM bank
            for tq in range(NT):
                nc.tensor.matmul(
                    score128[32 * tq : 32 * tq + 1, :],
                    lhsT=v_sb[:, lc : lc + 1],
                    rhs=tanh_tiles[lc][:, tq * 512 : (tq + 1) * 512],
                    start=(lc == 0),
                    stop=(lc == LC - 1),
                    tile_position=(0, 32 * tq),
                    skip_group_check=True,
                )

        def emit_softmax(b, score128, tanh_tiles):
            # remaining score chunks
            emit_score(b, LC - 2, score128, tanh_tiles)
            emit_score(b, LC - 1, score128, tanh_tiles)

            # --- softmax over T (partition 0; exp on ScalarE) ---
            # No max subtraction: scores are O(1) sums against V ~ N(0,1/LD),
            # so exp cannot overflow and softmax(s) == softmax(s - max)
            # exactly. Each exp starts as soon as its score quarter stops.
            arow = rowp.tile([1, T], BF16, tag="arow", name=f"arow_{b}")
            sm4 = smallp.tile([1, NT], F32, tag="sm4", name=f"sm4_{b}")
            for tq in range(NT):
                sl = slice(tq * 512, (tq + 1) * 512)
                nc.scalar.activation(
                    out=arow[0:1, sl],
                    in_=score128[32 * tq : 32 * tq + 1, :],
                    func=AF.Exp,
                    bias=0.0,
                    scale=1.0,
                    accum_out=sm4[0:1, tq : tq + 1],
                )
            ssum = smallp.tile([1, 1], F32, tag="ssum", name=f"ssum_{b}")
            nc.vector.reduce_sum(out=ssum, in_=sm4, axis=AX.X)
            rsum = smallp.tile([1, 1], F32, tag="rsum", name=f"rsum_{b}")
            nc.vector.reciprocal(rsum, ssum)
            return arow, rsum

        def emit_context(b, arow, rsum, enc_tiles):
            # normalized f32 attn row for the output (ScalarE, off the
            # critical path — runs while DVE does the context reduction)
            anorm = rowp.tile([1, T], F32, tag="anorm", name=f"anorm_{b}")
            nc.scalar.activation(
                out=anorm[0:1, :], in_=arow[0:1, :], func=AF.Copy,
                scale=rsum[0:1, :],
            )
            nc.sync.dma_start(out=out_attn[b : b + 1, :], in_=anorm[0:1, :])

            # --- context[h] = sum_t attn[t] * enc[h, t] ---
            # broadcast the UNnormalized attn row to 128 partitions on the
            # PE (ones.T @ arow — starts as soon as each exp quarter lands);
            # the 1/sum scale is applied to the tiny context result at the
            # end, using an exact f32 broadcast of rsum via a DRAM
            # round-trip that hides under the context reduction
            rs_dram = dramp.tile([1, 1], F32, tag="rs_dram", name=f"rs_dram_{b}")
            nc.sync.dma_start(out=rs_dram, in_=rsum)
            rs_bc = smallp.tile([128, 1], F32, tag="rs_bc", name=f"rs_bc_{b}")
            nc.sync.dma_start(out=rs_bc, in_=rs_dram[0:1, :].partition_broadcast(128))
            abc_ps = []
            for tq in range(NT):
                aps = pssc.tile(
                    [128, 512], F32, tag="score", name=f"abc_ps_{b}_{tq}"
                )
                nc.tensor.matmul(
                    aps,
                    lhsT=ones128,
                    rhs=arow[0:1, tq * 512 : (tq + 1) * 512],
                    start=True,
                    stop=True,
                )
                abc_ps.append(aps)
            abc = bcp.tile([128, T], BF16, tag="abc", name=f"abc_{b}")
            for tq in range(NT):
                nc.vector.tensor_copy(
                    abc[:, tq * 512 : (tq + 1) * 512], abc_ps[tq]
                )
            ctxs = smallp.tile([128, HC], F32, tag="ctx", name=f"ctxs_{b}")
            for hc in range(HC):
                # scr = (enc bypass) * attn_u ; ctxs[:,hc] = sum_t scr
                scr = bigp.tile([128, T], BF16, tag="big", name=f"scr_{b}_{hc}")
                nc.vector.scalar_tensor_tensor(
                    out=scr,
                    in0=enc_tiles[hc],
                    scalar=1.0,
                    in1=abc,
                    op0=ALU.bypass,
                    op1=ALU.mult,
                    accum_out=ctxs[:, hc : hc + 1],
                )
            nc.vector.tensor_scalar_mul(ctxs, ctxs, rs_bc[:, 0:1])
            # device-natural [p, hc] layout: contiguous per-partition DMA
            # (the host untangles h = hc*128 + p)
            nc.sync.dma_start(out=out_ctx[b], in_=ctxs)

        # --- main per-batch pipeline ---
        pending_softmax = None
        pending_context = None
        pending_context_args = None
        for b in range(BL):
            if b == 0:
                enc_tiles = enc0_tiles
            else:
                enc_tiles = []
                for hc in range(HC):
                    et = encp.tile([128, T], BF16, tag="enc", name=f"enc_b{b}_h{hc}")
                    nc.sync.dma_start(
                        out=et, in_=enc_t[b, hc * 128 : (hc + 1) * 128, :]
                    )
                    enc_tiles.append(et)

            score128 = None
            # main matmuls: enc_w[l, t] accumulated over h; one weight load
            # (W2 h,l block) covers 4 moving matmuls.  One PSUM bank per
            # t-quarter so ScalarE evacuation releases slots incrementally.
            tanh_tiles = []
            for lc in range(LC):
                psq = [
                    psmm.tile([128, 512], F32, tag="encw", name=f"encw_b{b}_l{lc}q{q}")
                    for q in range(4)
                ]
                for hc in range(HC):
                    for q in range(4):
                        nc.tensor.matmul(
                            psq[q],
                            lhsT=w2_sb[:, hc, lc * 128 : (lc + 1) * 128],
                            rhs=enc_tiles[hc][:, q * 512 : (q + 1) * 512],
                            start=(hc == 0),
                            stop=(hc == HC - 1),
                        )
                if lc == 0 and pending_softmax is not None:
                    # previous batch's softmax: emitted here so its exp runs
                    # on ScalarE right after that batch's own tanh quarters,
                    # before this batch's tanh
                    pending_context_args = pending_softmax()
                    pending_softmax = None
                if lc == 1 and pending_context is not None:
                    # previous batch's broadcast + context: by now its exp
                    # results are ready, so the PE broadcast doesn't stall
                    pending_context(*pending_context_args)
                    pending_context = None
                th = tanhp.tile([128, T], BF16, tag="tanh", name=f"tanh_b{b}_l{lc}")
                tanh_tiles.append(th)
                for q in range(4):
                    # tanh(dec_w * enc_w + dec_w*b2), per-partition scale/bias
                    nc.scalar.activation(
                        out=th[:, q * 512 : (q + 1) * 512],
                        in_=psq[q],
                        func=AF.Tanh,
                        scale=dec_w[:, lc, b : b + 1],
                        bias=db2[:, lc, b : b + 1],
                    )
                if lc >= 2:
                    # scores for lc-2 (its tanh is ready by now); the score
                    # PSUM tile is allocated lazily here so its slot isn't
                    # held across the previous batch's deferred epilogue
                    if score128 is None:
                        score128 = pssc.tile(
                            [128, 512], F32, tag="score", name=f"score_b{b}"
                        )
                    emit_score(b, lc - 2, score128, tanh_tiles)

            pending_softmax = (
                lambda b=b, sp=score128, tt=tanh_tiles:
                emit_softmax(b, sp, tt)
            )
            def _mk_ctx(b=b, et=enc_tiles):
                return lambda arow, rsum: emit_context(b, arow, rsum, et)
            pending_context = _mk_ctx()
        pending_context_args = pending_softmax()
        pending_context(*pending_context_args)
    return nc


def _get_nc():
    if "nc" not in _CACHE:
        _CACHE["nc"] = _build()
    return _CACHE["nc"]


def kernel(encoder_hidden, decoder_hidden, W1, b1, W2, b2, V, bV, _trace=False):
    encoder_hidden = np.asarray(encoder_hidden, dtype=np.float32)
    decoder_hidden = np.asarray(decoder_hidden, dtype=np.float32)
    W1 = np.asarray(W1, dtype=np.float32)
    W2 = np.asarray(W2, dtype=np.float32)
    b1 = np.asarray(b1, dtype=np.float32)
    b2 = np.asarray(b2, dtype=np.float32)
    V = np.asarray(V, dtype=np.float32)

    # host-side prep: relayout + bf16 wire format (the device computes in
    # bf16 either way — this is the same rounding the on-device DMA cast
    # performed, just done host-side so the wire bytes halve)
    import ml_dtypes

    bf16 = ml_dtypes.bfloat16
    enc_t_full = np.ascontiguousarray(
        encoder_hidden.transpose(0, 2, 1).astype(bf16)
    )  # [B, H, T] bf16
    b2t = np.ascontiguousarray(b2.reshape(LC, 128).T)                     # [128, LC]
    W1b = W1.astype(bf16)
    W2b = W2.astype(bf16)
    b1b = b1.astype(bf16)
    # v and decoder pre-arranged into their on-chip [partition, free]
    # layouts so the loads are clean contiguous descriptors
    Vp = np.ascontiguousarray(V.reshape(LC, 128).T.astype(bf16))          # [128, LC]

    in_maps = []
    for c in range(NCORES):
        sl = slice(c * BL, (c + 1) * BL)
        in_maps.append(
            {
                "enc_t": np.ascontiguousarray(enc_t_full[sl]),
                "dec_p": np.ascontiguousarray(
                    decoder_hidden[sl]                       # [BL, H]
                    .T.reshape(HC, 128, BL)                  # [hc, p, b]
                    .transpose(1, 0, 2)                      # [p, hc, b]
                    .reshape(128, HC * BL)
                    .astype(bf16)
                ),
                "w1": W1b,
                "w2": W2b,
                "b1": b1b,
                "b2t": b2t,
                "v": Vp,
            }
        )

    nc = _get_nc()
    res = run_bass_kernel_spmd(nc, in_maps, core_ids=list(range(NCORES)), trace=_trace)
    if _trace:
        _CACHE["last_result"] = res

    context = np.concatenate(
        [r["out_ctx"].transpose(0, 2, 1).reshape(BL, H) for r in res.results],
        axis=0,
    )  # [B, H]  (h = hc*128 + p)
    attn = np.concatenate([r["out_attn"] for r in res.results], axis=0)    # [B, T]
    return context.astype(np.float32), attn.reshape(B, T, 1).astype(np.float32)
